# revision 1
# baseline (speedup 1.0000x reference)
"""Trainium2 Bass kernel for DiffSelfAttention (B=1, T=2048, C=2048, 16 v-heads).

Sharding: tensor-parallel over heads across 8 NeuronCores. Core c owns
v-heads {2c, 2c+1} plus the matching q/k heads of both differential branches.
Each core computes its qkv slice, the attention for its 4 q/k heads, the
differential + per-head RMSNorm, and a partial projection
y_c = out_c @ w_proj[rows_c]. The host sums the 8 partials (unshard step).

Layout/strategy notes:
  - All matmuls run as float32r (full-rate fp32 on the PE at N>=256,
    ~2e-4 element rounding). DMA loads go directly into fp32r tiles;
    on-chip fp32r operands are produced by compute ops (engines round on
    write), which is what the BIR verifier requires.
  - q/k are produced directly transposed ([d, T]); v in natural layout
    ([T, d]); scores computed transposed ([tk, tq]) so probs@v needs no
    transposes anywhere.
  - Softmax divisions are eliminated: RMSNorm is invariant to any
    per-column positive scale, so instead of a1/r1 - lam*a2/r2 we feed it
    o' = a1*r2 - lam*a2*r1 (r = exp-sum broadcasts from a ones-matmul).
    The 1e-6 RMS eps is dropped: mean(o'^2) >> eps always for this data.
  - rsqrt for RMS is computed as exp(-0.5*log(m)) on the ACT engine
    (Reciprocal/Rsqrt activations are banned; Log+Exp share one ACT
    table set so there are no mid-kernel table switches).
  - Softmax column sums use two interleaved DVE accumulator chains so the
    serial dependency never gates the ACT exp stream.
"""

import math

import numpy as np

import concourse.bass as bass
import concourse.bacc as bacc
import concourse.mybir as mybir
import concourse.tile as tile

F32 = mybir.dt.float32
F32R = mybir.dt.float32r

T = 2048
C = 2048
N_HEAD = 16
H_DIM = 64
D2 = 2 * H_DIM  # 128 (v-head dim, also the RMS group size)
LAMBDA_INIT = 0.8 - 0.6 * math.exp(-0.3)
SCALE = 1.0 / math.sqrt(H_DIM)
P = 128
KSLABS = C // P  # 16 contraction slabs
TT = T // P  # 16 t-tiles
NCH = 512  # moving-operand chunk (max for 4-byte dtypes)
HQ = T // 2  # 1024-wide tq halves in the attention inner loop
N_CORES = 8

EXP = mybir.ActivationFunctionType.Exp
LOG = mybir.ActivationFunctionType.Ln
MULT = mybir.AluOpType.mult
ADD = mybir.AluOpType.add


def build(lam: float) -> bass.Bass:
    nc = bacc.Bacc("TRN2", target_bir_lowering=False, debug=False)

    xt_d = nc.dram_tensor("xt", [P, 4, KSLABS, NCH], F32R, kind="ExternalInput")
    wqk_d = nc.dram_tensor("wqk", [P, KSLABS, 4 * P], F32R, kind="ExternalInput")
    wv_d = nc.dram_tensor("wv", [P, KSLABS, 2 * D2], F32R, kind="ExternalInput")
    wp_d = nc.dram_tensor("wp", [P, 2, T], F32R, kind="ExternalInput")
    sv_d = nc.dram_tensor("sv", [P, 1], F32, kind="ExternalInput")
    y_d = nc.dram_tensor("y", [TT, P, T], F32, kind="ExternalOutput")

    with tile.TileContext(nc) as tc:
        with tc.tile_pool(name="persist", bufs=1) as persist:
            sv = persist.tile([P, 1], F32)
            ones_f = persist.tile([P, P], F32)
            ones = persist.tile([P, P], F32R)
            qk = persist.tile([P, 4, T], F32R)  # q1|q2|k1|k2, [d, T] layout
            vnat = persist.tile([P, TT, 2 * D2], F32R)  # v, [T, d] layout
            nc.sync.dma_start(out=sv, in_=sv_d[:])
            nc.vector.memset(ones_f, 1.0)
            nc.vector.tensor_copy(ones, ones_f)

            # ---------- phase 1: qkv projections ----------
            with tc.tile_pool(name="w1", bufs=1) as w1p, \
                 tc.tile_pool(name="xt", bufs=2) as xtp, \
                 tc.tile_pool(name="ps_qk", bufs=2, space="PSUM") as pqk, \
                 tc.tile_pool(name="ps_v", bufs=2, space="PSUM") as pvp:
                wqk = w1p.tile([P, KSLABS, 4 * P], F32R)
                wv = w1p.tile([P, KSLABS, 2 * D2], F32R)
                nc.sync.dma_start(out=wqk, in_=wqk_d[:])
                nc.sync.dma_start(out=wv, in_=wv_d[:])
                for n in range(T // NCH):  # 512-wide t chunks
                    xt = xtp.tile([P, KSLABS, NCH], F32R)
                    nc.sync.dma_start(out=xt, in_=xt_d[:, n, :, :])
                    for m in range(4):  # q1, q2, k1, k2
                        ps = pqk.tile([P, NCH], F32)
                        for k in range(KSLABS):
                            nc.tensor.matmul(
                                ps,
                                wqk[:, k, m * P:(m + 1) * P],
                                xt[:, k, :],
                                start=(k == 0),
                                stop=(k == KSLABS - 1),
                            )
                        nc.vector.tensor_copy(qk[:, m, n * NCH:(n + 1) * NCH], ps)
                    for t2 in range(NCH // P):  # t-tiles in this chunk
                        ps = pvp.tile([P, 2 * D2], F32)
                        for k in range(KSLABS):
                            nc.tensor.matmul(
                                ps,
                                xt[:, k, t2 * P:(t2 + 1) * P],
                                wv[:, k, :],
                                start=(k == 0),
                                stop=(k == KSLABS - 1),
                            )
                        nc.vector.tensor_copy(vnat[:, n * (NCH // P) + t2, :], ps)

            # ---------- phases 2+3 ----------
            with tc.tile_pool(name="wp", bufs=1) as wpp:
                wp = wpp.tile([P, 2, T], F32R)
                on = wpp.tile([P, 2, T], F32R)  # normed diff out, [d, T] per vh
                nc.sync.dma_start(out=wp, in_=wp_d[:])

                # ---------- phase 2: attention ----------
                # Both v-head streams (array rows 0-63 / 64-127) are packed
                # into shared [P, 2, NCH] tiles: one ACT exp covers both, and
                # the PE gets 6 matmuls per tk-slab (scores x2, pv x2,
                # colsum x2) so it never idles long enough for the HAM
                # clock-gate to re-throttle it to 1.2 GHz.
                with tc.tile_pool(name="ps_s", bufs=2, space="PSUM") as psp, \
                     tc.tile_pool(name="ps_a", bufs=1, space="PSUM") as pap, \
                     tc.tile_pool(name="ps_r", bufs=1, space="PSUM") as rp, \
                     tc.tile_pool(name="exp", bufs=4) as ep, \
                     tc.tile_pool(name="keep", bufs=1) as kp:
                    opk = kp.tile([P, 2, T], F32)  # scaled diff o', per vh
                    a1u = {}
                    r1l = {}
                    for br in range(2):
                        for q4 in range(4):  # 512-wide tq quarters
                            c0 = q4 * NCH
                            pa = pap.tile([P, 2, NCH], F32, tag="pa")
                            r = rp.tile([P, 2, NCH], F32, tag="r")
                            for k in range(TT):  # tk slabs
                                ps = psp.tile([P, 2, NCH], F32, tag="s")
                                et = ep.tile([P, 2, NCH], F32R, tag="er")
                                for vh in range(2):
                                    rows = slice(vh * H_DIM, (vh + 1) * H_DIM)
                                    nc.tensor.matmul(
                                        ps[:, vh, :],
                                        qk[rows, 2 + br, k * P:(k + 1) * P],
                                        qk[rows, br, c0:c0 + NCH],
                                        start=True,
                                        stop=True,
                                    )
                                nc.scalar.activation(et, ps, EXP, scale=SCALE)
                                for vh in range(2):
                                    nc.tensor.matmul(
                                        pa[:, vh, :],
                                        vnat[:, k, vh * D2:(vh + 1) * D2],
                                        et[:, vh, :],
                                        start=(k == 0),
                                        stop=(k == TT - 1),
                                    )
                                    nc.tensor.matmul(
                                        r[:, vh, :],
                                        ones,
                                        et[:, vh, :],
                                        start=(k == 0),
                                        stop=(k == TT - 1),
                                    )
                            if br == 0:
                                # keep unnormalized a1 and -lam*r1 for branch 2
                                a1u[q4] = kp.tile([P, 2, NCH], F32, tag=f"a1u{q4}", name=f"a1u{q4}")
                                nc.vector.tensor_copy(a1u[q4], pa)
                                r1l[q4] = kp.tile([P, 2, NCH], F32, tag=f"r1l{q4}", name=f"r1l{q4}")
                                nc.vector.tensor_scalar_mul(r1l[q4], r, -lam)
                            else:
                                # o' = a1*r2 - lam*a2*r1  (a per-column positive
                                # rescale of o; RMSNorm cancels it)
                                m1 = ep.tile([P, 2, NCH], F32, tag="m1")
                                nc.vector.tensor_mul(m1, a1u[q4], r)
                                m2 = ep.tile([P, 2, NCH], F32, tag="m2")
                                nc.vector.tensor_mul(m2, pa, r1l[q4])
                                nc.vector.tensor_add(opk[:, :, c0:c0 + NCH], m1, m2)
                    # RMS: rsqrt(mean o'^2) = exp(-0.5*ln(mean)). All Ln ops
                    # emitted before all Exp ops -> at most 2 ACT table loads.
                    psms = []
                    for vh in range(2):
                        for hf in range(2):
                            q0 = hf * HQ
                            sq = ep.tile([P, HQ], F32R, tag="er")
                            nc.vector.tensor_mul(sq, opk[:, vh, q0:q0 + HQ], opk[:, vh, q0:q0 + HQ])
                            psm = psp.tile([P, HQ], F32, tag="s")
                            for c2 in range(2):
                                nc.tensor.matmul(
                                    psm[:, c2 * NCH:(c2 + 1) * NCH],
                                    ones,
                                    sq[:, c2 * NCH:(c2 + 1) * NCH],
                                    start=True,
                                    stop=True,
                                )
                            ln = kp.tile([P, HQ], F32, tag=f"a1u{2 * vh + hf}", name=f"ln{vh}{hf}")
                            nc.scalar.activation(ln, psm, LOG, scale=1.0 / D2)
                            psms.append(ln)
                    for vh in range(2):
                        for hf in range(2):
                            q0 = hf * HQ
                            rsq = ep.tile([P, HQ], F32, tag="m1")
                            nc.scalar.activation(rsq, psms[2 * vh + hf], EXP, scale=-0.5)
                            nc.vector.scalar_tensor_tensor(
                                on[:, vh, q0:q0 + HQ],
                                opk[:, vh, q0:q0 + HQ],
                                sv, rsq, op0=MULT, op1=MULT,
                            )

                # ---------- phase 3: output projection (partial sum) ----------
                with tc.tile_pool(name="ps_y", bufs=4, space="PSUM") as pyp, \
                     tc.tile_pool(name="ysb", bufs=3) as yp:
                    for tt_i in range(TT):
                        ysb = yp.tile([P, T], F32)
                        for nch in range(T // NCH):
                            py = pyp.tile([P, NCH], F32)
                            for vh in range(2):
                                nc.tensor.matmul(
                                    py,
                                    on[:, vh, tt_i * P:(tt_i + 1) * P],
                                    wp[:, vh, nch * NCH:(nch + 1) * NCH],
                                    start=(vh == 0),
                                    stop=(vh == 1),
                                )
                            nc.vector.tensor_copy(ysb[:, nch * NCH:(nch + 1) * NCH], py)
                        nc.sync.dma_start(out=y_d[tt_i], in_=ysb)
    nc.finalize()
    return nc


def _core_inputs(x, w_qkv, w_proj, rms_scale):
    """Host-side shard prep: per-core weight slices + replicated x^T."""
    xt = np.ascontiguousarray(x.reshape(T, C).T)  # [C, T]
    xtr = np.ascontiguousarray(
        xt.reshape(KSLABS, P, T // NCH, NCH).transpose(1, 2, 0, 3)
    )
    sv = np.ascontiguousarray(
        (rms_scale.astype(np.float32) * np.float32(1.0 - LAMBDA_INIT)).reshape(P, 1)
    )
    maps = []
    for c in range(N_CORES):
        cols = [
            w_qkv[:, 0 * 1024 + c * P:0 * 1024 + (c + 1) * P],  # q1 heads 2c,2c+1
            w_qkv[:, 1 * 1024 + c * P:1 * 1024 + (c + 1) * P],  # q2
            w_qkv[:, 2 * 1024 + c * P:2 * 1024 + (c + 1) * P],  # k1
            w_qkv[:, 3 * 1024 + c * P:3 * 1024 + (c + 1) * P],  # k2
        ]
        wqk = np.concatenate(cols, axis=1)  # [C, 512]
        wqk = np.ascontiguousarray(wqk.reshape(KSLABS, P, 4 * P).transpose(1, 0, 2))
        wv = w_qkv[:, 2 * C + c * 2 * D2:2 * C + (c + 1) * 2 * D2]  # [C, 256]
        wv = np.ascontiguousarray(wv.reshape(KSLABS, P, 2 * D2).transpose(1, 0, 2))
        wp = w_proj[c * 2 * D2:(c + 1) * 2 * D2, :]  # [256, T]
        wp = np.ascontiguousarray(wp.reshape(2, P, T).transpose(1, 0, 2))
        maps.append({"xt": xtr, "wqk": wqk, "wv": wv, "wp": wp, "sv": sv})
    return maps


def kernel(x, w_qkv, w_proj, lambda_q1, lambda_k1, lambda_q2, lambda_k2, rms_scale):
    from concourse.bass_utils import run_bass_kernel_spmd

    x = np.asarray(x, dtype=np.float32)
    w_qkv = np.asarray(w_qkv, dtype=np.float32)
    w_proj = np.asarray(w_proj, dtype=np.float32)
    rms_scale = np.asarray(rms_scale, dtype=np.float32)
    lam1 = np.exp(np.sum(np.asarray(lambda_q1) * np.asarray(lambda_k1), dtype=np.float32))
    lam2 = np.exp(np.sum(np.asarray(lambda_q2) * np.asarray(lambda_k2), dtype=np.float32))
    lam = float(lam1 - lam2 + LAMBDA_INIT)

    nc = build(lam)
    in_maps = _core_inputs(x, w_qkv, w_proj, rms_scale)
    res = run_bass_kernel_spmd(nc, in_maps, core_ids=list(range(N_CORES)))
    y = np.zeros((TT, P, T), np.float32)
    for rmap in res.results:
        y += rmap["y"]
    return y.reshape(1, T, C)



# revision 20
# speedup vs baseline: 1.2778x; 1.2778x over previous
"""Trainium2 Bass kernel for DiffSelfAttention (B=1, T=2048, C=2048, 16 v-heads).

Sharding: tensor-parallel over heads across 8 NeuronCores. Core c owns
v-heads {2c, 2c+1} plus the matching q/k heads of both differential branches.
Each core computes its qkv slice, the attention for its 4 q/k head-pairs, the
differential + per-head RMSNorm, and a partial projection
y_c = out_c @ w_proj[rows_c]. The host sums the 8 partials (unshard step).

v2 design notes (vs the fp32r v1):
  - Everything bf16 on the PE (1 cycle/row at ANY moving size, halved DMA
    and SBUF footprint). Host converts inputs; rel-err budget is 2e-2 and
    bf16 keeps us ~1e-2 or better.
  - Transposed PV: attention is computed as a^T[tq,d2] = et^T @ [v|1] with
    the exp'd scores as the STATIONARY operand and [v | ones-column] as a
    129-wide moving operand. This gets the softmax denominator r in the
    same matmul (column 128) AND puts r on the partition axis, so all the
    differential-combine scalars are per-partition [P,1] operands — no
    broadcast matmuls. v1's separate ones-colsum (131k cycles/core) is gone.
  - Softmax divisions eliminated as in v1: RMSNorm is invariant to any
    per-column positive scale, so o' = a1*r2 - lam*a2*r1 feeds the norm.
  - RMS rsqrt = exp(-0.5*ln(mean)) on ACT, batched [P,8] per (block,head).
    rms_scale * (1-lambda_init) is folded into w_proj rows on the host.
  - o'[tq,d2] is transposed back to [d2,tq] for the projection with the
    DMA xbar transpose engine (idle mid-kernel), not the PE.
  - Work is emitted in 2 tq-blocks of 1024; the q2 projections for the
    second half and the first block's output projection are injected as
    fillers into the (ACT-bound) attention sweeps so the PE never idles.
  - PSUM budget (8 banks): scores [P,1024]x2 = 4, pv accumulators
    (8 x [P,132] packed 3-per-bank) = 3, proj/filler = 1.
"""

import math

import numpy as np

import concourse.bass as bass
import concourse.bacc as bacc
import concourse.mybir as mybir
import concourse.tile as tile

F32 = mybir.dt.float32
BF16 = mybir.dt.bfloat16

T = 2048
C = 2048
N_HEAD = 16
H_DIM = 64
D2 = 2 * H_DIM  # 128 (v-head dim, also the RMS group size)
LAMBDA_INIT = 0.8 - 0.6 * math.exp(-0.3)
SCALE = 1.0 / math.sqrt(H_DIM)
P = 128
KS = C // P  # 16 contraction slabs
TT = T // P  # 16 t-tiles
NCH = 512  # phase-1 t-chunk width
QS = 8  # tq slabs per block
NBLK = 2  # tq blocks of 1024
N_CORES = 8

EXP = mybir.ActivationFunctionType.Exp
LOG = mybir.ActivationFunctionType.Ln
CPY = mybir.ActivationFunctionType.Copy
MULT = mybir.AluOpType.mult
ADD = mybir.AluOpType.add


def build(lam: float) -> bass.Bass:
    nc = bacc.Bacc("TRN2", target_bir_lowering=False, debug=False)

    xb_d = nc.dram_tensor("xt", [P, 4, KS, NCH], BF16, kind="ExternalInput")
    wqk_d = nc.dram_tensor("wqk", [P, KS, 4 * P], BF16, kind="ExternalInput")
    wv_d = nc.dram_tensor("wv", [P, KS, 2 * D2], BF16, kind="ExternalInput")
    wp_d = nc.dram_tensor("wp", [P, 2, T], BF16, kind="ExternalInput")
    id_d = nc.dram_tensor("ident", [P, P], BF16, kind="ExternalInput")
    y_d = nc.dram_tensor("y", [TT, P, T], BF16, kind="ExternalOutput")

    with tile.TileContext(nc) as tc:
        with tc.tile_pool(name="persist", bufs=1) as pp, \
             tc.tile_pool(name="etp", bufs=3) as etp, \
             tc.tile_pool(name="work", bufs=2) as wkp, \
             tc.tile_pool(name="ysp", bufs=2) as ysp, \
             tc.tile_pool(name="sc", bufs=2, space="PSUM") as scp, \
             tc.tile_pool(name="acc", bufs=1, space="PSUM") as accp, \
             tc.tile_pool(name="yp", bufs=1, space="PSUM") as ypp:

            xb = pp.tile([P, 4, KS, NCH], BF16)
            wqk = pp.tile([P, KS, 4 * P], BF16)
            wv = pp.tile([P, KS, 2 * D2], BF16)
            wp = pp.tile([P, 2, T], BF16)
            qk = pp.tile([P, 4, T], BF16)  # m: q1|q2|k1|k2, [d, T] layout
            ident = pp.tile([P, P], BF16)
            vb = pp.tile([P, KS, 2, 130], BF16)  # [tk, kslab, vh, v|1|pad]

            nc.sync.dma_start(out=wqk, in_=wqk_d[:])
            nc.sync.dma_start(out=xb[:, 0], in_=xb_d[:, 0])
            nc.sync.dma_start(out=wv, in_=wv_d[:])
            nc.sync.dma_start(out=xb[:, 1], in_=xb_d[:, 1])
            nc.sync.dma_start(out=xb[:, 2], in_=xb_d[:, 2])
            nc.sync.dma_start(out=xb[:, 3], in_=xb_d[:, 3])
            nc.sync.dma_start(out=wp, in_=wp_d[:])
            nc.sync.dma_start(out=ident, in_=id_d[:])
            nc.gpsimd.memset(vb[:, :, :, D2:D2 + 1], 1.0)

            # ---------------- phase 1: qkv projections ----------------
            # (PSUM->SBUF copies ride the ACT engine, idle until the first
            # exp; GPSIMD/Pool cannot read PSUM)
            def emit_qkv_m(n, m):
                ps = scp.tile([P, NCH], F32, tag="s", name=f"psq{n}{m}")
                for k in range(KS):
                    nc.tensor.matmul(
                        ps,
                        wqk[:, k, m * P:(m + 1) * P],
                        xb[:, n, k, :],
                        start=(k == 0),
                        stop=(k == KS - 1),
                    )
                nc.scalar.activation(qk[:, m, n * NCH:(n + 1) * NCH], ps, CPY)

            def emit_v(n, t2):
                g = 4 * n + t2
                ps = scp.tile([P, 2, D2], F32, tag="s", name=f"psv{g}")
                for k in range(KS):
                    nc.tensor.matmul(
                        ps,
                        xb[:, n, k, t2 * P:(t2 + 1) * P],
                        wv[:, k, :],
                        start=(k == 0),
                        stop=(k == KS - 1),
                    )
                nc.scalar.activation(vb[:, g, :, 0:D2], ps, CPY)

            for n in range(4):
                for m in (2, 3):  # k1, k2 (stationaries for all sweeps)
                    emit_qkv_m(n, m)
                for t2 in range(4):
                    emit_v(n, t2)
            for n in (0, 1):
                for m in (0, 1):  # q1, q2 for block 0
                    emit_qkv_m(n, m)

            # ---- filler machinery: PE work injected into ACT-bound sweeps
            q_fillers = []  # q projections for tq-block 1 (drain in block 0)
            proj_fillers = []  # block-0 output projection (drain in block 1)

            def inject(budget_ns):
                while (q_fillers or proj_fillers) and budget_ns > 0:
                    est, f = (q_fillers or proj_fillers).pop(0)
                    f()
                    budget_ns -= est

            def queue_q_fillers(n, m):
                # q projections for tq-block 1, chunk n, using the yp psum
                # slot (idle during block-0 sweeps)
                box = {}

                def mk(k):
                    def f():
                        if k == 0:
                            box["ps"] = ypp.tile(
                                [P, NCH], F32, tag="y", name=f"psq{n}{m}"
                            )
                        nc.tensor.matmul(
                            box["ps"],
                            wqk[:, k, m * P:(m + 1) * P],
                            xb[:, n, k, :],
                            start=(k == 0),
                            stop=(k == KS - 1),
                        )
                        if k == KS - 1:
                            nc.vector.tensor_copy(
                                qk[:, m, n * NCH:(n + 1) * NCH], box["ps"]
                            )

                    return (220.0, f)

                q_fillers.extend(mk(k) for k in range(KS))

            for n in (2, 3):
                for m in (0, 1):
                    queue_q_fillers(n, m)

            def emit_tp(o2, vh, q, otT, psum_src=None, act_copy=False):
                pool, tag = psum_src if psum_src else (scp, "s")
                pt = pool.tile([P, P], BF16, tag=tag, name=f"tp{vh}{q}")
                nc.tensor.transpose(pt, o2[:, q, :], ident)
                if act_copy:
                    nc.scalar.activation(otT[:, vh, q, :], pt, CPY)
                else:
                    nc.vector.tensor_copy(otT[:, vh, q, :], pt)

            def queue_tp_pairs(o2, vh, otT):
                for q0 in range(0, QS, 2):
                    def f(q0=q0):
                        emit_tp(o2, vh, q0, otT)
                        emit_tp(o2, vh, q0 + 1, otT)
                    q_fillers.append((120.0, f))

            # ---------------- phase 2: attention sweeps ----------------
            def get_accs(bk, vh, br):
                a = accp.tile([P, 3, 132], F32, tag="accA", name=f"accA{bk}{vh}{br}")
                b = accp.tile([P, 3, 132], F32, tag="accB", name=f"accB{bk}{vh}{br}")
                c = accp.tile([P, 2, 132], F32, tag="accC", name=f"accC{bk}{vh}{br}")
                return [(a, 0), (a, 1), (a, 2), (b, 0), (b, 1), (b, 2), (c, 0), (c, 1)]

            def sweep(bk, vh, br):
                # scores + exp + pv for head-pair (vh,br), tq block bk,
                # software-pipelined one k-slab ahead so the PE never waits
                # a full exp latency
                rows = slice(vh * H_DIM, (vh + 1) * H_DIM)
                accs = get_accs(bk, vh, br)
                ets = [None] * TT
                for j in range(TT + 1):
                    if j < TT:
                        ps = scp.tile(
                            [P, 2 * NCH], F32, tag="s", name=f"sc{bk}{vh}{br}{j}"
                        )
                        for i in range(2):
                            nc.tensor.matmul(
                                ps[:, i * NCH:(i + 1) * NCH],
                                qk[rows, 2 + br, j * P:(j + 1) * P],
                                qk[rows, br, bk * 1024 + i * NCH:
                                   bk * 1024 + (i + 1) * NCH],
                                start=True,
                                stop=True,
                            )
                        et = etp.tile(
                            [P, 2 * NCH], BF16, tag="e", name=f"et{bk}{vh}{br}{j}"
                        )
                        nc.scalar.activation(et, ps, EXP, scale=SCALE)
                        ets[j] = et
                    if j > 0:
                        inject(900.0 if j == 1 else 220.0)
                        et = ets[j - 1]
                        for q in range(QS):
                            at, qi = accs[q]
                            # one psum accumulation group per BANK: start
                            # zeroes the whole 2KB zero-region lazily
                            first = (j - 1 == 0) and qi == 0
                            lastq = qi == (3 if q < 6 else 2) - 1
                            nc.tensor.matmul(
                                at[:, qi, 0:129],
                                et[:, q * P:(q + 1) * P],
                                vb[:, j - 1, vh, 0:129],
                                start=first,
                                stop=(j - 1 == TT - 1) and lastq,
                            )
                        ets[j - 1] = None
                return accs

            def save_accs(bk, vh, br, accs, tag):
                # copy a|r psum accumulators to SBUF (frees the acc banks
                # for the next sweep after ~3 DVE ops)
                sb = wkp.tile([P, QS, 132], F32, tag=tag, name=f"{tag}{bk}{vh}")
                nc.vector.tensor_copy(sb[:, 0:3, 0:129], accs[0][0][:, :, 0:129])
                nc.vector.tensor_copy(sb[:, 3:6, 0:129], accs[3][0][:, :, 0:129])
                nc.vector.tensor_copy(sb[:, 6:8, 0:129], accs[6][0][:, :, 0:129])
                return sb

            def combine(bk, vh, asb, bsb, otT, eager_rms=False):
                # o' = a1*r2 - lam*a2*r1 (per-column rescale of the true o;
                # RMSNorm cancels it), then per-head RMS + bf16 + transpose.
                r1n = wkp.tile([P, QS, 1], F32, tag="r1n", name=f"r1n{bk}{vh}")
                o12 = wkp.tile([P, QS, P], F32, tag="o12", name=f"o12{bk}{vh}")
                sqs = wkp.tile([P, P], F32, tag="sqs", name=f"sqs{bk}{vh}")
                msb = wkp.tile([P, QS, 1], F32, tag="msb", name=f"msb{bk}{vh}")
                lns = wkp.tile([P, QS, 1], F32, tag="lns", name=f"lns{bk}{vh}")
                rs = wkp.tile([P, QS, 1], F32, tag="rs", name=f"rs{bk}{vh}")
                o2 = wkp.tile([P, QS, P], BF16, tag="o2", name=f"o2{bk}{vh}")
                nc.vector.tensor_scalar_mul(r1n, asb[:, :, 128:129], -lam)

                def rms_tail(qs):
                    nc.scalar.activation(
                        lns[:, qs, :], msb[:, qs, :], LOG, scale=1.0 / D2
                    )
                    nc.scalar.activation(
                        rs[:, qs, :], lns[:, qs, :], EXP, scale=-0.5
                    )

                for q in range(QS):
                    nc.vector.tensor_scalar_mul(
                        o12[:, q, :], asb[:, q, 0:P], bsb[:, q, 128:129]
                    )
                    nc.vector.scalar_tensor_tensor(
                        o12[:, q, :], bsb[:, q, 0:P], r1n[:, q, :], o12[:, q, :],
                        op0=MULT, op1=ADD,
                    )
                    nc.vector.tensor_mul(sqs, o12[:, q, :], o12[:, q, :])
                    nc.vector.tensor_reduce(
                        msb[:, q, :], sqs, mybir.AxisListType.X, ADD
                    )
                    if eager_rms:
                        rms_tail(slice(q, q + 1))
                        nc.vector.tensor_scalar_mul(
                            o2[:, q, :], o12[:, q, :], rs[:, q, :]
                        )
                        emit_tp(o2, vh, q, otT,
                                psum_src=tail_rot[q % 4],
                                act_copy=(q % 2 == 1))
                if not eager_rms:
                    rms_tail(slice(0, QS))
                    for q in range(QS):
                        nc.vector.tensor_scalar_mul(
                            o2[:, q, :], o12[:, q, :], rs[:, q, :]
                        )
                    queue_tp_pairs(o2, vh, otT)

            # -------- output projection for one 128-row tq tile ----------
            # psum_src: (pool, tag) for this tile's psum bank. Mid-kernel
            # (filler path) only the 1-bank "y" tag is free and copies go to
            # DVE; at the tail the attention accumulator banks are stolen
            # for a 4-bank rotation and copies split DVE/ACT.
            def proj_tile_closures(otT, t, psum_src, split_copies):
                q = t % QS
                pool, tag = psum_src
                box = {}
                cl = []

                def c_vh0(p):
                    def f():
                        if p == 0:
                            box["yp"] = pool.tile([P, 2, 256], F32, tag=tag,
                                                  name=f"yt{t}")
                            box["ys"] = ysp.tile([P, 4, 2, 256], BF16,
                                                 tag="ysb", name=f"ys{t}")
                        yp = box["yp"]
                        for r in range(2):
                            nc.tensor.matmul(
                                yp[:, r, :],
                                otT[:, 0, q, :],
                                wp[:, 0, 512 * p + 256 * r:512 * p + 256 * (r + 1)],
                                start=(r == 0),
                                stop=False,
                            )
                    return (230.0, f)

                def c_vh1(p):
                    def f():
                        yp = box["yp"]
                        for r in range(2):
                            nc.tensor.matmul(
                                yp[:, r, :],
                                otT[:, 1, q, :],
                                wp[:, 1, 512 * p + 256 * r:512 * p + 256 * (r + 1)],
                                start=False,
                                stop=(r == 1),
                            )
                        if split_copies:
                            nc.vector.tensor_copy(box["ys"][:, p, 0, :],
                                                  yp[:, 0, :])
                            nc.scalar.activation(box["ys"][:, p, 1, :],
                                                 yp[:, 1, :], CPY)
                        else:
                            nc.vector.tensor_copy(box["ys"][:, p, :, :], yp)
                    return (230.0, f)

                def c_dma():
                    nc.sync.dma_start(out=y_d[t], in_=box["ys"])

                for p in range(4):
                    cl.append(c_vh0(p))
                    cl.append(c_vh1(p))
                cl.append((0.0, c_dma))
                return cl

            # ---------------- blocks ----------------
            tail_rot = [(ypp, "y"), (accp, "accA"), (accp, "accB"),
                        (accp, "accC")]
            for bk in range(NBLK):
                if bk == 1:
                    # block-1 scores read the filler-produced q projections:
                    # force-drain any q fillers that block 0 didn't absorb
                    while q_fillers:
                        q_fillers.pop(0)[1]()
                otT = wkp.tile([P, 2, QS, P], BF16, tag="otT", name=f"otT{bk}")
                last = bk == NBLK - 1
                for vh in range(2):
                    accs0 = sweep(bk, vh, 0)
                    asb = save_accs(bk, vh, 0, accs0, "asb")
                    accs1 = sweep(bk, vh, 1)
                    bsb = save_accs(bk, vh, 1, accs1, "bsb")
                    combine(bk, vh, asb, bsb, otT,
                            eager_rms=(last and vh == 1))
                if not last:
                    for t in range(QS):
                        proj_fillers.extend(
                            proj_tile_closures(otT, bk * QS + t, (ypp, "y"),
                                               split_copies=False))
                else:
                    inject(1e9)  # drain leftovers
                    for t in range(QS):
                        for est, f in proj_tile_closures(
                                otT, bk * QS + t, tail_rot[t % 4],
                                split_copies=True):
                            f()
    nc.finalize()
    return nc


def _core_inputs(x, w_qkv, w_proj, rms_scale):
    """Host-side shard prep: per-core bf16 weight slices + replicated x^T."""
    bf = mybir.dt.np(BF16)
    ident = np.ascontiguousarray(np.eye(P, dtype=np.float32).astype(bf))
    xt = x.reshape(T, C).T  # [C, T]
    xtr = np.ascontiguousarray(
        xt.reshape(KS, P, 4, NCH).transpose(1, 2, 0, 3).astype(bf)
    )
    sv = np.tile(
        rms_scale.astype(np.float32) * np.float32(1.0 - LAMBDA_INIT), 2
    )  # [256], per-row scale for this core's w_proj rows
    maps = []
    for c in range(N_CORES):
        cols = [
            w_qkv[:, 0 * 1024 + c * P:0 * 1024 + (c + 1) * P],  # q1 heads 2c,2c+1
            w_qkv[:, 1 * 1024 + c * P:1 * 1024 + (c + 1) * P],  # q2
            w_qkv[:, 2 * 1024 + c * P:2 * 1024 + (c + 1) * P],  # k1
            w_qkv[:, 3 * 1024 + c * P:3 * 1024 + (c + 1) * P],  # k2
        ]
        wqk = np.concatenate(cols, axis=1)  # [C, 512]
        wqk = np.ascontiguousarray(
            wqk.reshape(KS, P, 4 * P).transpose(1, 0, 2).astype(bf)
        )
        wv = w_qkv[:, 2 * C + c * 2 * D2:2 * C + (c + 1) * 2 * D2]  # [C, 256]
        wv = np.ascontiguousarray(
            wv.reshape(KS, P, 2 * D2).transpose(1, 0, 2).astype(bf)
        )
        wp = w_proj[c * 2 * D2:(c + 1) * 2 * D2, :] * sv[:, None]  # [256, T]
        wp = np.ascontiguousarray(
            wp.reshape(2, P, T).transpose(1, 0, 2).astype(bf)
        )
        maps.append({"xt": xtr, "wqk": wqk, "wv": wv, "wp": wp, "ident": ident})
    return maps


def kernel(x, w_qkv, w_proj, lambda_q1, lambda_k1, lambda_q2, lambda_k2, rms_scale):
    from concourse.bass_utils import run_bass_kernel_spmd

    x = np.asarray(x, dtype=np.float32)
    w_qkv = np.asarray(w_qkv, dtype=np.float32)
    w_proj = np.asarray(w_proj, dtype=np.float32)
    rms_scale = np.asarray(rms_scale, dtype=np.float32)
    lam1 = np.exp(np.sum(np.asarray(lambda_q1) * np.asarray(lambda_k1), dtype=np.float32))
    lam2 = np.exp(np.sum(np.asarray(lambda_q2) * np.asarray(lambda_k2), dtype=np.float32))
    lam = float(lam1 - lam2 + LAMBDA_INIT)

    nc = build(lam)
    in_maps = _core_inputs(x, w_qkv, w_proj, rms_scale)
    res = run_bass_kernel_spmd(nc, in_maps, core_ids=list(range(N_CORES)))
    y = np.zeros((TT, P, T), np.float32)
    for rmap in res.results:
        y += np.asarray(rmap["y"], np.float32)
    return y.reshape(1, T, C)


# revision 21
# speedup vs baseline: 1.3792x; 1.0794x over previous
"""Trainium2 Bass kernel for DiffSelfAttention (B=1, T=2048, C=2048, 16 v-heads).

Sharding: tensor-parallel over heads across 8 NeuronCores. Core c owns
v-heads {2c, 2c+1} plus the matching q/k heads of both differential branches.
Each core computes its qkv slice, the attention for its 4 q/k head-pairs, the
differential + per-head RMSNorm, and a partial projection
y_c = out_c @ w_proj[rows_c]. The host sums the 8 partials (unshard step).

v2 design notes (vs the fp32r v1):
  - Everything bf16 on the PE (1 cycle/row at ANY moving size, halved DMA
    and SBUF footprint). Host converts inputs; rel-err budget is 2e-2 and
    bf16 keeps us ~1e-2 or better.
  - Transposed PV: attention is computed as a^T[tq,d2] = et^T @ [v|1] with
    the exp'd scores as the STATIONARY operand and [v | ones-column] as a
    129-wide moving operand. This gets the softmax denominator r in the
    same matmul (column 128) AND puts r on the partition axis, so all the
    differential-combine scalars are per-partition [P,1] operands — no
    broadcast matmuls. v1's separate ones-colsum (131k cycles/core) is gone.
  - Softmax divisions eliminated as in v1: RMSNorm is invariant to any
    per-column positive scale, so o' = a1*r2 - lam*a2*r1 feeds the norm.
  - RMS rsqrt = exp(-0.5*ln(mean)) on ACT, batched [P,8] per (block,head).
    rms_scale * (1-lambda_init) is folded into w_proj rows on the host.
  - o'[tq,d2] is transposed back to [d2,tq] for the projection with the
    DMA xbar transpose engine (idle mid-kernel), not the PE.
  - Work is emitted in 2 tq-blocks of 1024; the q2 projections for the
    second half and the first block's output projection are injected as
    fillers into the (ACT-bound) attention sweeps so the PE never idles.
  - PSUM budget (8 banks): scores [P,1024]x2 = 4, pv accumulators
    (8 x [P,132] packed 3-per-bank) = 3, proj/filler = 1.
"""

import math

import numpy as np

import concourse.bass as bass
import concourse.bacc as bacc
import concourse.mybir as mybir
import concourse.tile as tile

F32 = mybir.dt.float32
BF16 = mybir.dt.bfloat16

T = 2048
C = 2048
N_HEAD = 16
H_DIM = 64
D2 = 2 * H_DIM  # 128 (v-head dim, also the RMS group size)
LAMBDA_INIT = 0.8 - 0.6 * math.exp(-0.3)
SCALE = 1.0 / math.sqrt(H_DIM)
P = 128
KS = C // P  # 16 contraction slabs
TT = T // P  # 16 t-tiles
NCH = 512  # phase-1 t-chunk width
QS = 8  # tq slabs per block
NBLK = 2  # tq blocks of 1024
N_CORES = 8

EXP = mybir.ActivationFunctionType.Exp
LOG = mybir.ActivationFunctionType.Ln
CPY = mybir.ActivationFunctionType.Copy
MULT = mybir.AluOpType.mult
ADD = mybir.AluOpType.add


def build(lam: float) -> bass.Bass:
    nc = bacc.Bacc("TRN2", target_bir_lowering=False, debug=False)

    xb_d = nc.dram_tensor("xt", [P, 4, KS, NCH], BF16, kind="ExternalInput")
    wqk_d = nc.dram_tensor("wqk", [P, 4, KS, P], BF16, kind="ExternalInput")
    wv_d = nc.dram_tensor("wv", [P, KS, 2 * D2], BF16, kind="ExternalInput")
    wp_d = nc.dram_tensor("wp", [P, 2, T], BF16, kind="ExternalInput")
    id_d = nc.dram_tensor("ident", [P, P], BF16, kind="ExternalInput")
    y_d = nc.dram_tensor("y", [TT, P, T], BF16, kind="ExternalOutput")

    with tile.TileContext(nc) as tc:
        with tc.tile_pool(name="persist", bufs=1) as pp, \
             tc.tile_pool(name="etp", bufs=3) as etp, \
             tc.tile_pool(name="work", bufs=2) as wkp, \
             tc.tile_pool(name="ysp", bufs=2) as ysp, \
             tc.tile_pool(name="sc", bufs=2, space="PSUM") as scp, \
             tc.tile_pool(name="acc", bufs=1, space="PSUM") as accp, \
             tc.tile_pool(name="yp", bufs=1, space="PSUM") as ypp:

            xb = pp.tile([P, 4, KS, NCH], BF16)
            wqk = pp.tile([P, 4, KS, P], BF16)
            wv = pp.tile([P, KS, 2 * D2], BF16)
            wp = pp.tile([P, 2, T], BF16)
            qk = pp.tile([P, 4, T], BF16)  # m: q1|q2|k1|k2, [d, T] layout
            ident = pp.tile([P, P], BF16)
            vb = pp.tile([P, KS, 2, 130], BF16)  # [tk, kslab, vh, v|1|pad]

            nc.sync.dma_start(out=wqk[:, 2], in_=wqk_d[:, 2])  # k1 first
            nc.sync.dma_start(out=wqk[:, 3], in_=wqk_d[:, 3])
            nc.sync.dma_start(out=xb[:, 0], in_=xb_d[:, 0])
            nc.sync.dma_start(out=wv, in_=wv_d[:])
            nc.sync.dma_start(out=wqk[:, 0], in_=wqk_d[:, 0])
            nc.sync.dma_start(out=wqk[:, 1], in_=wqk_d[:, 1])
            nc.sync.dma_start(out=xb[:, 1], in_=xb_d[:, 1])
            nc.sync.dma_start(out=xb[:, 2], in_=xb_d[:, 2])
            nc.sync.dma_start(out=xb[:, 3], in_=xb_d[:, 3])
            nc.sync.dma_start(out=wp, in_=wp_d[:])
            nc.sync.dma_start(out=ident, in_=id_d[:])
            nc.gpsimd.memset(vb[:, :, :, D2:D2 + 1], 1.0)

            # ---------------- phase 1: qkv projections ----------------
            # (PSUM->SBUF copies ride the ACT engine, idle until the first
            # exp; GPSIMD/Pool cannot read PSUM)
            def emit_qkv_m(n, m):
                ps = scp.tile([P, NCH], F32, tag="s", name=f"psq{n}{m}")
                for k in range(KS):
                    nc.tensor.matmul(
                        ps,
                        wqk[:, m, k, :],
                        xb[:, n, k, :],
                        start=(k == 0),
                        stop=(k == KS - 1),
                    )
                nc.scalar.activation(qk[:, m, n * NCH:(n + 1) * NCH], ps, CPY)

            def emit_v(n, t2):
                g = 4 * n + t2
                ps = scp.tile([P, 2, D2], F32, tag="s", name=f"psv{g}")
                for k in range(KS):
                    nc.tensor.matmul(
                        ps,
                        xb[:, n, k, t2 * P:(t2 + 1) * P],
                        wv[:, k, :],
                        start=(k == 0),
                        stop=(k == KS - 1),
                    )
                nc.scalar.activation(vb[:, g, :, 0:D2], ps, CPY)

            for n in range(4):
                for m in (2, 3):  # k1, k2 (stationaries for all sweeps)
                    emit_qkv_m(n, m)
                for t2 in range(4):
                    emit_v(n, t2)
            for n in (0, 1):
                for m in (0, 1):  # q1, q2 for block 0
                    emit_qkv_m(n, m)

            # ---- filler machinery: PE work injected into ACT-bound sweeps
            q_fillers = []  # q projections for tq-block 1 (drain in block 0)
            tp_fillers = []  # deferred PE transposes (gated: DVE deps lag)
            proj_fillers = []  # block-0 output projection (drain in block 1)

            def inject(budget_ns, tp_ok=True):
                while budget_ns > 0:
                    if q_fillers:
                        lst = q_fillers
                    elif tp_fillers:
                        if not tp_ok:
                            return  # keep ordering: proj waits for tps
                        lst = tp_fillers
                    elif proj_fillers:
                        lst = proj_fillers
                    else:
                        return
                    est, f = lst.pop(0)
                    f()
                    budget_ns -= est

            def queue_q_fillers(n, m):
                # q projections for tq-block 1, chunk n, using the yp psum
                # slot (idle during block-0 sweeps)
                box = {}

                def mk(k):
                    def f():
                        if k == 0:
                            box["ps"] = ypp.tile(
                                [P, NCH], F32, tag="y", name=f"psq{n}{m}"
                            )
                        nc.tensor.matmul(
                            box["ps"],
                            wqk[:, m, k, :],
                            xb[:, n, k, :],
                            start=(k == 0),
                            stop=(k == KS - 1),
                        )
                        if k == KS - 1:
                            nc.vector.tensor_copy(
                                qk[:, m, n * NCH:(n + 1) * NCH], box["ps"]
                            )

                    return (220.0, f)

                q_fillers.extend(mk(k) for k in range(KS))

            for n in (2, 3):
                for m in (0, 1):
                    queue_q_fillers(n, m)

            def emit_tp(o2, vh, q, otT, psum_src=None, act_copy=False):
                pool, tag = psum_src if psum_src else (scp, "s")
                pt = pool.tile([P, P], BF16, tag=tag, name=f"tp{vh}{q}")
                nc.tensor.transpose(pt, o2[:, q, :], ident)
                if act_copy:
                    nc.scalar.activation(otT[:, vh, q, :], pt, CPY)
                else:
                    nc.vector.tensor_copy(otT[:, vh, q, :], pt)

            def queue_tp_pairs(o2, vh, otT):
                for q0 in range(0, QS, 2):
                    def f(q0=q0):
                        emit_tp(o2, vh, q0, otT)
                        emit_tp(o2, vh, q0 + 1, otT)
                    tp_fillers.append((120.0, f))

            # ---------------- phase 2: attention sweeps ----------------
            def get_accs(bk, vh, br):
                a = accp.tile([P, 3, 132], F32, tag="accA", name=f"accA{bk}{vh}{br}")
                b = accp.tile([P, 3, 132], F32, tag="accB", name=f"accB{bk}{vh}{br}")
                c = accp.tile([P, 2, 132], F32, tag="accC", name=f"accC{bk}{vh}{br}")
                return [(a, 0), (a, 1), (a, 2), (b, 0), (b, 1), (b, 2), (c, 0), (c, 1)]

            def sweep(bk, vh, br):
                # scores + exp + pv for head-pair (vh,br), tq block bk,
                # software-pipelined one k-slab ahead so the PE never waits
                # a full exp latency
                rows = slice(vh * H_DIM, (vh + 1) * H_DIM)
                accs = get_accs(bk, vh, br)
                ets = [None] * TT
                for j in range(TT + 1):
                    if j < TT:
                        ps = scp.tile(
                            [P, 2 * NCH], F32, tag="s", name=f"sc{bk}{vh}{br}{j}"
                        )
                        for i in range(2):
                            nc.tensor.matmul(
                                ps[:, i * NCH:(i + 1) * NCH],
                                qk[rows, 2 + br, j * P:(j + 1) * P],
                                qk[rows, br, bk * 1024 + i * NCH:
                                   bk * 1024 + (i + 1) * NCH],
                                start=True,
                                stop=True,
                            )
                        et = etp.tile(
                            [P, 2 * NCH], BF16, tag="e", name=f"et{bk}{vh}{br}{j}"
                        )
                        nc.scalar.activation(et, ps, EXP, scale=SCALE)
                        ets[j] = et
                    if j > 0:
                        inject(900.0 if j == 1 else 220.0, tp_ok=(j >= 8))
                        et = ets[j - 1]
                        for q in range(QS):
                            at, qi = accs[q]
                            # one psum accumulation group per BANK: start
                            # zeroes the whole 2KB zero-region lazily
                            first = (j - 1 == 0) and qi == 0
                            lastq = qi == (3 if q < 6 else 2) - 1
                            nc.tensor.matmul(
                                at[:, qi, 0:129],
                                et[:, q * P:(q + 1) * P],
                                vb[:, j - 1, vh, 0:129],
                                start=first,
                                stop=(j - 1 == TT - 1) and lastq,
                            )
                        ets[j - 1] = None
                return accs

            def save_accs(bk, vh, br, accs, tag):
                # copy a|r psum accumulators to SBUF (frees the acc banks
                # for the next sweep after ~3 DVE ops)
                sb = wkp.tile([P, QS, 132], F32, tag=tag, name=f"{tag}{bk}{vh}")
                nc.vector.tensor_copy(sb[:, 0:3, 0:129], accs[0][0][:, :, 0:129])
                nc.vector.tensor_copy(sb[:, 3:6, 0:129], accs[3][0][:, :, 0:129])
                nc.vector.tensor_copy(sb[:, 6:8, 0:129], accs[6][0][:, :, 0:129])
                return sb

            def combine(bk, vh, asb, bsb, otT):
                # o' = a1*r2 - lam*a2*r1 (per-column rescale of the true o;
                # RMSNorm cancels it), then per-head RMS + bf16 + transpose.
                r1n = wkp.tile([P, QS, 1], F32, tag="r1n", name=f"r1n{bk}{vh}")
                o12 = wkp.tile([P, QS, P], F32, tag="o12", name=f"o12{bk}{vh}")
                sqs = wkp.tile([P, P], F32, tag="sqs", name=f"sqs{bk}{vh}")
                msb = wkp.tile([P, QS, 1], F32, tag="msb", name=f"msb{bk}{vh}")
                lns = wkp.tile([P, QS, 1], F32, tag="lns", name=f"lns{bk}{vh}")
                rs = wkp.tile([P, QS, 1], F32, tag="rs", name=f"rs{bk}{vh}")
                o2 = wkp.tile([P, QS, P], BF16, tag="o2", name=f"o2{bk}{vh}")
                nc.vector.tensor_scalar_mul(r1n, asb[:, :, 128:129], -lam)

                def rms_tail(qs):
                    nc.scalar.activation(
                        lns[:, qs, :], msb[:, qs, :], LOG, scale=1.0 / D2
                    )
                    nc.scalar.activation(
                        rs[:, qs, :], lns[:, qs, :], EXP, scale=-0.5
                    )

                for q in range(QS):
                    nc.vector.tensor_scalar_mul(
                        o12[:, q, :], asb[:, q, 0:P], bsb[:, q, 128:129]
                    )
                    nc.vector.scalar_tensor_tensor(
                        o12[:, q, :], bsb[:, q, 0:P], r1n[:, q, :], o12[:, q, :],
                        op0=MULT, op1=ADD,
                    )
                    nc.vector.tensor_mul(sqs, o12[:, q, :], o12[:, q, :])
                    nc.vector.tensor_reduce(
                        msb[:, q, :], sqs, mybir.AxisListType.X, ADD
                    )
                rms_tail(slice(0, QS))
                for q in range(QS):
                    nc.vector.tensor_scalar_mul(
                        o2[:, q, :], o12[:, q, :], rs[:, q, :]
                    )
                queue_tp_pairs(o2, vh, otT)

            # -------- output projection for one 128-row tq tile ----------
            # psum_src: (pool, tag) for this tile's psum bank. Mid-kernel
            # (filler path) only the 1-bank "y" tag is free and copies go to
            # DVE; at the tail the attention accumulator banks are stolen
            # for a 4-bank rotation and copies split DVE/ACT.
            def proj_tile_closures(otT, t, psum_src, split_copies):
                q = t % QS
                pool, tag = psum_src
                box = {}
                cl = []

                def c_vh0(p):
                    def f():
                        if p == 0:
                            box["yp"] = pool.tile([P, 2, 256], F32, tag=tag,
                                                  name=f"yt{t}")
                            box["ys"] = ysp.tile([P, 4, 2, 256], BF16,
                                                 tag="ysb", name=f"ys{t}")
                        yp = box["yp"]
                        for r in range(2):
                            nc.tensor.matmul(
                                yp[:, r, :],
                                otT[:, 0, q, :],
                                wp[:, 0, 512 * p + 256 * r:512 * p + 256 * (r + 1)],
                                start=(r == 0),
                                stop=False,
                            )
                    return (230.0, f)

                def c_vh1(p):
                    def f():
                        yp = box["yp"]
                        for r in range(2):
                            nc.tensor.matmul(
                                yp[:, r, :],
                                otT[:, 1, q, :],
                                wp[:, 1, 512 * p + 256 * r:512 * p + 256 * (r + 1)],
                                start=False,
                                stop=(r == 1),
                            )
                        if split_copies:
                            nc.vector.tensor_copy(box["ys"][:, p, 0, :],
                                                  yp[:, 0, :])
                            nc.scalar.activation(box["ys"][:, p, 1, :],
                                                 yp[:, 1, :], CPY)
                        else:
                            nc.vector.tensor_copy(box["ys"][:, p, :, :], yp)
                    return (230.0, f)

                def c_dma():
                    nc.sync.dma_start(out=y_d[t], in_=box["ys"])

                for p in range(4):
                    cl.append(c_vh0(p))
                    cl.append(c_vh1(p))
                cl.append((0.0, c_dma))
                return cl

            # ---------------- blocks ----------------
            tail_rot = [(ypp, "y"), (accp, "accA"), (accp, "accB"),
                        (accp, "accC")]
            for bk in range(NBLK):
                if bk == 1:
                    # block-1 scores read the filler-produced q projections:
                    # force-drain any q fillers that block 0 didn't absorb
                    while q_fillers:
                        q_fillers.pop(0)[1]()
                otT = wkp.tile([P, 2, QS, P], BF16, tag="otT", name=f"otT{bk}")
                last = bk == NBLK - 1
                for vh in range(2):
                    accs0 = sweep(bk, vh, 0)
                    asb = save_accs(bk, vh, 0, accs0, "asb")
                    accs1 = sweep(bk, vh, 1)
                    bsb = save_accs(bk, vh, 1, accs1, "bsb")
                    combine(bk, vh, asb, bsb, otT)
                if not last:
                    for t in range(QS):
                        proj_fillers.extend(
                            proj_tile_closures(otT, bk * QS + t, (ypp, "y"),
                                               split_copies=False))
                else:
                    inject(1e9)  # drain leftovers (incl. final transposes)
                    for t in range(QS):
                        for est, f in proj_tile_closures(
                                otT, bk * QS + t, tail_rot[t % 4],
                                split_copies=True):
                            f()
    nc.finalize()
    return nc


def _core_inputs(x, w_qkv, w_proj, rms_scale):
    """Host-side shard prep: per-core bf16 weight slices + replicated x^T."""
    bf = mybir.dt.np(BF16)
    ident = np.ascontiguousarray(np.eye(P, dtype=np.float32).astype(bf))
    xt = x.reshape(T, C).T  # [C, T]
    xtr = np.ascontiguousarray(
        xt.reshape(KS, P, 4, NCH).transpose(1, 2, 0, 3).astype(bf)
    )
    sv = np.tile(
        rms_scale.astype(np.float32) * np.float32(1.0 - LAMBDA_INIT), 2
    )  # [256], per-row scale for this core's w_proj rows
    maps = []
    for c in range(N_CORES):
        cols = [
            w_qkv[:, 0 * 1024 + c * P:0 * 1024 + (c + 1) * P],  # q1 heads 2c,2c+1
            w_qkv[:, 1 * 1024 + c * P:1 * 1024 + (c + 1) * P],  # q2
            w_qkv[:, 2 * 1024 + c * P:2 * 1024 + (c + 1) * P],  # k1
            w_qkv[:, 3 * 1024 + c * P:3 * 1024 + (c + 1) * P],  # k2
        ]
        wqk = np.stack(cols, axis=0)  # [4, C, 128]
        wqk = np.ascontiguousarray(
            wqk.reshape(4, KS, P, P).transpose(2, 0, 1, 3).astype(bf)
        )
        wv = w_qkv[:, 2 * C + c * 2 * D2:2 * C + (c + 1) * 2 * D2]  # [C, 256]
        wv = np.ascontiguousarray(
            wv.reshape(KS, P, 2 * D2).transpose(1, 0, 2).astype(bf)
        )
        wp = w_proj[c * 2 * D2:(c + 1) * 2 * D2, :] * sv[:, None]  # [256, T]
        wp = np.ascontiguousarray(
            wp.reshape(2, P, T).transpose(1, 0, 2).astype(bf)
        )
        maps.append({"xt": xtr, "wqk": wqk, "wv": wv, "wp": wp, "ident": ident})
    return maps


def kernel(x, w_qkv, w_proj, lambda_q1, lambda_k1, lambda_q2, lambda_k2, rms_scale):
    from concourse.bass_utils import run_bass_kernel_spmd

    x = np.asarray(x, dtype=np.float32)
    w_qkv = np.asarray(w_qkv, dtype=np.float32)
    w_proj = np.asarray(w_proj, dtype=np.float32)
    rms_scale = np.asarray(rms_scale, dtype=np.float32)
    lam1 = np.exp(np.sum(np.asarray(lambda_q1) * np.asarray(lambda_k1), dtype=np.float32))
    lam2 = np.exp(np.sum(np.asarray(lambda_q2) * np.asarray(lambda_k2), dtype=np.float32))
    lam = float(lam1 - lam2 + LAMBDA_INIT)

    nc = build(lam)
    in_maps = _core_inputs(x, w_qkv, w_proj, rms_scale)
    res = run_bass_kernel_spmd(nc, in_maps, core_ids=list(range(N_CORES)))
    y = np.zeros((TT, P, T), np.float32)
    for rmap in res.results:
        y += np.asarray(rmap["y"], np.float32)
    return y.reshape(1, T, C)
